# revision 16
# baseline (speedup 1.0000x reference)
"""KNN (k=10, mode vote over 100 classes) on 8 Trainium2 cores.

Strategy: shard the reference set `data`/`targets` across 8 cores along N
(6250 rows each, padded to 6400). Each core computes, for every query q and
local point n, the score  s[q,n] = 2*X[q]@d[n] + (512 - ||d[n]||^2)  (monotone
in -dist^2 per query; +512 centers scores near 0 for fp16 fidelity).

Matmuls are fp8e4m3 DoubleRow (K=256 per instruction, streaming at the same
~217ns/512-col pace as a K=128 fp16 matmul -> 2x MAC throughput). The bias
rides inside the second contraction chunk: chunk1 = dims 0..255; chunk2 =
dims 256..509 on partitions 0..126 plus the fp8 bias and its fp8 residual on
partition 127 (query side carries 1.0 there). Dims 510/511 are dropped from
the device score (noise sigma ~2.8, audited harmless). Two matmuls per
128-query x 512-point tile.

Candidate extraction is hierarchical: ScalarE copies PSUM->SBUF as dense
fp16; VectorE tensor_reduce (2x 16-bit mode) computes the max of every
16-wide segment; GpSimd merges segment maxes into packed fp32 words
(fp16 segmax << 16 | segment index, IEEE order = lexicographic); VectorE
max8 then returns the top-8 segments of each 2048-wide unit with their
indices in one short pass. A unit's top-8 segments provably contain its
top-8 elements, and no unit holds >8 of a query's true top-10 (audited:
max 5, worst in-unit device rank 4).

Host merges 8 cores x 4 units x 8 = 256 candidate segments per query and
rescores exactly in fp64 with sound adaptive pruning: after rescoring the
top-16 segments by segmax, any unscored segment whose segmax (an upper bound
on members' device scores) is below the current 10th-best exact score minus
the device-error margin cannot hold a true top-10 point.
"""

from contextlib import ExitStack

import numpy as np
import ml_dtypes

import concourse.bacc as bacc
import concourse.bass as bass
import concourse.mybir as mybir
from concourse.bass_utils import run_bass_kernel_spmd
from concourse.tile import TileContext

F32 = mybir.dt.float32
F16 = mybir.dt.float16
FP8 = mybir.dt.float8e4
U16 = mybir.dt.uint16
COPY = mybir.ActivationFunctionType.Copy
DR = mybir.MatmulPerfMode.DoubleRow
MAX = mybir.AluOpType.max
AX = mybir.AxisListType.X

Q = 1024            # queries
D = 512             # feature dim
N = 50000           # reference points
CORES = 8
NSH = N // CORES    # 6250 per core
NPAD = 6400         # padded shard width
K = 10
NUM_CLASSES = 100
SUBW = 512          # matmul free-dim tile (one PSUM bank)
SEG = 32
NSEG = NPAD // SEG  # 200 segments per row
UNITS = [(0, 2048), (2048, 2048), (4096, 2048), (6144, 256)]
NCAND = len(UNITS) * 8   # 32 candidate segments per core per query
QT = Q // 128
NBUF = 3
DELTA = 24.0        # device-score error margin for sound host pruning


def build_program() -> bass.Bass:
    nc = bacc.Bacc()
    xq = nc.declare_dram_parameter("xq", [128, 4, Q], FP8, isOutput=False)
    dq = nc.declare_dram_parameter("dq", [128, 4, NPAD], FP8, isOutput=False)
    vals_o = nc.declare_dram_parameter("vals", [128, QT * NCAND], F32, isOutput=True)

    with TileContext(nc) as tc, ExitStack() as ctx:
        const = ctx.enter_context(tc.tile_pool(name="const", bufs=1))
        ppool = ctx.enter_context(tc.tile_pool(name="ppool", bufs=2, space="PSUM"))

        # spread input DMAs across engine DGE rings so transfers overlap;
        # order = first-use order so unit 0 computes while the rest stream in
        rings = [nc.sync, nc.scalar, nc.gpsimd]
        xt = const.tile([128, 4, Q], FP8, tag="xt", name="xt")
        rings[0].dma_start(xt[:], xq[:])
        dts = {}
        for g, (goff, gw) in enumerate(UNITS):
            for c in range(2):
                t = const.tile([128, 2, gw], FP8, tag=f"dt{g}_{c}", name=f"dt{g}_{c}")
                rings[(2 * g + c + 1) % 3].dma_start(
                    t[:], dq[:, 2 * c : 2 * c + 2, goff : goff + gw]
                )
                dts[(g, c)] = t

        cvall = const.tile([128, QT * NCAND], F32, tag="cvall", name="cvall")

        sc16, sgm, sgp = [], [], []
        for i in range(NBUF):
            t = const.tile([128, NPAD], F16, tag=f"sc{i}", name=f"sc{i}")
            sc16.append(t)
            t = const.tile([128, NSEG], F16, tag=f"sgm{i}", name=f"sgm{i}")
            sgm.append(t)
            t = const.tile([128, NSEG], F32, tag=f"sgp{i}", name=f"sgp{i}")
            nc.gpsimd.iota(
                t.bitcast(U16)[:, 0 : 2 * NSEG : 2],
                pattern=[[1, NSEG]],
                base=0,
                channel_multiplier=0,
            )
            sgp.append(t)

        # PE warm-up during the DMA lead-in: ~20 junk matmuls on xt keep the
        # PE HAM busy so the real matmuls start at the 2.4 GHz clock
        for r in range(20):
            wp = ppool.tile([128, 2048], F32, tag="pp")
            nc.tensor.matmul(
                wp[:, :512], xt[:, 0:2, :128], xt[:, 0:2, :512],
                start=True, stop=True, perf_mode=DR,
            )

        for qt in range(QT):
            b = qt % NBUF
            for g, (goff, gw) in enumerate(UNITS):
                s0, s1 = goff // SEG, (goff + gw) // SEG
                pp = ppool.tile([128, 2048], F32, tag="pp")
                nsub = (gw + SUBW - 1) // SUBW
                for s in range(nsub):
                    w = min(SUBW, gw - s * SUBW)
                    out_sl = pp[:, s * SUBW : s * SUBW + w]
                    for c in range(2):
                        nc.tensor.matmul(
                            out_sl,
                            xt[:, 2 * c : 2 * c + 2, qt * 128 : (qt + 1) * 128],
                            dts[(g, c)][:, :, s * SUBW : s * SUBW + w],
                            start=(c == 0), stop=(c == 1), perf_mode=DR,
                        )
                    # PSUM -> dense fp16, per 2 banks; unit 2's second half
                    # goes to the otherwise-idle VectorE to balance ScalarE
                    if w < SUBW or s % 2 == 1:
                        lo = (s // 2) * 2 * SUBW
                        dst = sc16[b][:, goff + lo : goff + s * SUBW + w]
                        src = pp[:, lo : s * SUBW + w]
                        if g == 2 and s == 3:
                            nc.vector.tensor_copy(out=dst, in_=src)
                        else:
                            nc.scalar.activation(dst, src, COPY)
                # per-unit segment reduce only for the last qt (short tail);
                # earlier qts batch the reduce once per qt below
                if qt == QT - 1:
                    nc.vector.tensor_reduce(
                        sgm[b][:, s0:s1],
                        sc16[b][:, goff : goff + gw].rearrange(
                            "p (s e) -> p s e", e=SEG
                        ),
                        axis=AX, op=MAX,
                    )
                    nc.gpsimd.tensor_copy(
                        sgp[b].bitcast(F16)[:, 2 * s0 + 1 : 2 * s1 : 2],
                        sgm[b][:, s0:s1],
                    )
                    col = qt * NCAND + g * 8
                    nc.vector.max(
                        out=cvall[:, col : col + 8], in_=sgp[b][:, s0:s1]
                    )
            if qt < QT - 1:
                nc.vector.tensor_reduce(
                    sgm[b][:],
                    sc16[b][:].rearrange("p (s e) -> p s e", e=SEG),
                    axis=AX, op=MAX,
                )
                nc.gpsimd.tensor_copy(
                    sgp[b].bitcast(F16)[:, 1 : 2 * NSEG : 2], sgm[b][:]
                )
                for g, (goff, gw) in enumerate(UNITS):
                    s0, s1 = goff // SEG, (goff + gw) // SEG
                    col = qt * NCAND + g * 8
                    nc.vector.max(
                        out=cvall[:, col : col + 8], in_=sgp[b][:, s0:s1]
                    )
            # per-qt store: only the last (tiny) slice lands in the tail
            nc.gpsimd.dma_start(
                vals_o[:, qt * NCAND : (qt + 1) * NCAND],
                cvall[:, qt * NCAND : (qt + 1) * NCAND],
            )
    if not nc.is_finalized():
        nc.finalize()
    return nc


def _prep_inputs(X: np.ndarray, data: np.ndarray) -> list[dict[str, np.ndarray]]:
    e4 = ml_dtypes.float8_e4m3fn
    Xf = X.astype(np.float64)
    # query chunks: [p, 2c+s, q]; chunk1 ksub pair carries dims 256..509 on
    # partitions 0..126 and the constant 1.0 on partition 127 (bias rows)
    xqf = np.zeros((128, 4, Q), np.float64)
    xqf[:, 0, :] = (2.0 * Xf[:, 0:128]).T
    xqf[:, 1, :] = (2.0 * Xf[:, 128:256]).T
    xqf[:127, 2, :] = (2.0 * Xf[:, 256:383]).T
    xqf[:127, 3, :] = (2.0 * Xf[:, 383:510]).T
    xqf[127, 2, :] = 1.0
    xqf[127, 3, :] = 1.0
    xq8 = xqf.astype(e4)

    in_maps = []
    for i in range(CORES):
        sh = np.asarray(data[i * NSH : (i + 1) * NSH], dtype=np.float64)
        d2 = np.einsum("nd,nd->n", sh, sh)
        bias = np.full((NPAD,), -240.0, np.float64)
        bias[:NSH] = 512.0 - d2
        b0 = bias.astype(e4)
        b1 = np.where(
            np.arange(NPAD) < NSH, bias - b0.astype(np.float64), -240.0
        ).astype(e4)
        dqf = np.zeros((128, 4, NPAD), np.float64)
        dqf[:, 0, :NSH] = sh[:, 0:128].T
        dqf[:, 1, :NSH] = sh[:, 128:256].T
        dqf[:127, 2, :NSH] = sh[:, 256:383].T
        dqf[:127, 3, :NSH] = sh[:, 383:510].T
        dq8 = dqf.astype(e4)
        dq8[127, 2, :] = b0
        dq8[127, 3, :] = b1
        in_maps.append({"xq": xq8, "dq": dq8})
    return in_maps


def _merge(results, X, data, targets) -> np.ndarray:
    def unpack(a):  # [128, QT*NCAND] -> [Q, NCAND]
        return a.reshape(128, QT, NCAND).transpose(1, 0, 2).reshape(Q, NCAND)

    packed = np.stack(
        [unpack(results[i]["vals"]).view(np.uint32) for i in range(CORES)]
    )                                                      # [CORES, Q, NCAND]
    segidx = (packed & 0xFFFF).astype(np.int64)            # segment in shard row
    segmax = (packed >> 16).astype(np.uint16).view(np.float16).astype(np.float64)
    gseg = segidx + (np.arange(CORES, dtype=np.int64) * NSEG)[:, None, None]
    allv = segmax.transpose(1, 0, 2).reshape(Q, CORES * NCAND)
    alli = gseg.transpose(1, 0, 2).reshape(Q, CORES * NCAND)

    Xd = np.asarray(X, dtype=np.float64)
    dd = np.asarray(data, dtype=np.float64)
    tgt = np.asarray(targets, dtype=np.int64)

    def seg_cols(gs):
        core, seg = divmod(int(gs), NSEG)
        base = seg * SEG
        hi = min(base + SEG, NSH)
        if base >= NSH:
            return np.empty(0, np.int64)
        return core * NSH + np.arange(base, hi, dtype=np.int64)

    P1 = 16
    order = np.argsort(-allv, axis=1)
    pred = np.empty(Q, np.float32)
    counts = np.zeros(NUM_CLASSES, np.int32)
    for q in range(Q):
        segs1 = alli[q, order[q, :P1]]
        cols = np.concatenate([seg_cols(gs) for gs in segs1])
        sq = ((dd[cols] - Xd[q]) ** 2).sum(1)
        ord1 = np.argsort(sq, kind="stable")
        t10 = sq[ord1[min(K - 1, len(sq) - 1)]]            # 10th-best dist^2
        # s_dev ~ 512 + ||x||^2 - dist^2 (+/- DELTA device error): any segment
        # whose segmax is below this cannot hold a point within t10
        x2q = (Xd[q] ** 2).sum()
        thresh = (512.0 + x2q - t10) - DELTA
        rest = order[q, P1:]
        live = rest[allv[q, rest] >= thresh]
        if len(live):
            cols2 = np.concatenate([seg_cols(gs) for gs in alli[q, live]])
            if len(cols2):
                sq2 = ((dd[cols2] - Xd[q]) ** 2).sum(1)
                cols = np.concatenate([cols, cols2])
                sq = np.concatenate([sq, sq2])
        o = np.lexsort((cols, sq))[:K]
        top10 = cols[o]
        counts[:] = 0
        np.add.at(counts, tgt[top10], 1)
        pred[q] = counts.argmax()
    return pred


def kernel(X: np.ndarray, data: np.ndarray, targets: np.ndarray) -> np.ndarray:
    X = np.asarray(X)
    data = np.asarray(data)
    targets = np.asarray(targets)
    nc = build_program()
    in_maps = _prep_inputs(X, data)
    results = run_bass_kernel_spmd(nc, in_maps, list(range(CORES))).results
    return _merge(results, X, data, targets)


if __name__ == "__main__":
    import reference

    inputs = reference.setup_inputs()
    inputs = {k: np.asarray(v) for k, v in inputs.items()}
    out = kernel(**inputs)
    print(out[:16])


# revision 21
# speedup vs baseline: 1.2284x; 1.2284x over previous
"""KNN (k=10, mode vote over 100 classes) on 8 Trainium2 cores.

Strategy: shard the reference set `data`/`targets` across 8 cores along N
(6250 rows each, padded to 6400). Each core computes, for every query q and
local point n, the score  s[q,n] = 2*X[q]@d[n] + (512 - ||d[n]||^2)  (monotone
in -dist^2 per query; +512 centers scores near 0 for fp16 fidelity).

Matmuls are fp8e4m3 DoubleRow (K=256 per instruction, streaming at the same
~217ns/512-col pace as a K=128 fp16 matmul -> 2x MAC throughput). The bias
rides inside the second contraction chunk: chunk1 = dims 0..255; chunk2 =
dims 256..509 on partitions 0..126 plus the fp8 bias and its fp8 residual on
partition 127 (query side carries 1.0 there). Dims 510/511 are dropped from
the device score (noise sigma ~2.8, audited harmless). Two matmuls per
128-query x 512-point tile.

Candidate extraction is hierarchical: ScalarE copies PSUM->SBUF as dense
fp16; VectorE tensor_reduce (2x 16-bit mode) computes the max of every
16-wide segment; GpSimd merges segment maxes into packed fp32 words
(fp16 segmax << 16 | segment index, IEEE order = lexicographic); VectorE
max8 then returns the top-8 segments of each 2048-wide unit with their
indices in one short pass. A unit's top-8 segments provably contain its
top-8 elements, and no unit holds >8 of a query's true top-10 (audited:
max 5, worst in-unit device rank 4).

Host merges 8 cores x 4 units x 8 = 256 candidate segments per query and
rescores exactly in fp64 with sound adaptive pruning: after rescoring the
top-16 segments by segmax, any unscored segment whose segmax (an upper bound
on members' device scores) is below the current 10th-best exact score minus
the device-error margin cannot hold a true top-10 point.
"""

from contextlib import ExitStack

import numpy as np
import ml_dtypes

import concourse.bacc as bacc
import concourse.bass as bass
import concourse.mybir as mybir
from concourse.bass_utils import run_bass_kernel_spmd
from concourse.tile import TileContext

F32 = mybir.dt.float32
F16 = mybir.dt.float16
FP8 = mybir.dt.float8e4
U16 = mybir.dt.uint16
COPY = mybir.ActivationFunctionType.Copy
DR = mybir.MatmulPerfMode.DoubleRow
MAX = mybir.AluOpType.max
AX = mybir.AxisListType.X

Q = 1024            # queries
D = 512             # feature dim
N = 50000           # reference points
CORES = 8
NSH = N // CORES    # 6250 per core
NPAD = 6400         # padded shard width
K = 10
NUM_CLASSES = 100
SUBW = 512          # matmul free-dim tile (one PSUM bank)
SEG = 32
NSEG = NPAD // SEG  # 200 segments per row
UNITS = [(0, 2048), (2048, 2048), (4096, 2048), (6144, 256)]
NCAND = len(UNITS) * 8   # 32 candidate segments per core per query
QT = Q // 128
NBUF = 4
DELTA = 24.0        # device-score error margin for sound host pruning


def build_program() -> bass.Bass:
    nc = bacc.Bacc()
    xq = nc.declare_dram_parameter("xq", [128, 4, Q], FP8, isOutput=False)
    dq = nc.declare_dram_parameter("dq", [128, 4, NPAD], FP8, isOutput=False)
    vals_o = nc.declare_dram_parameter("vals", [128, QT * NCAND], F32, isOutput=True)

    with TileContext(nc) as tc, ExitStack() as ctx:
        const = ctx.enter_context(tc.tile_pool(name="const", bufs=1))
        ppool = ctx.enter_context(tc.tile_pool(name="ppool", bufs=2, space="PSUM"))

        # spread input DMAs across engine DGE rings so transfers overlap;
        # order = first-use order so unit 0 computes while the rest stream in
        # input DMAs split fine-grained, issued in first-use order across the
        # three DGE rings so unit 0's data lands within ~1us of kernel start
        rings = [nc.sync, nc.scalar, nc.gpsimd]
        ring_i = 0

        def dma(dst, src):
            nonlocal ring_i
            rings[ring_i % 3].dma_start(dst, src)
            ring_i += 1

        xt = const.tile([128, 4, Q], FP8, tag="xt", name="xt")
        dma(xt[:, 0:2, :], xq[:, 0:2, :])
        dts = {}
        for g, (goff, gw) in enumerate(UNITS):
            for c in range(2):
                t = const.tile([128, 2, gw], FP8, tag=f"dt{g}_{c}", name=f"dt{g}_{c}")
                dts[(g, c)] = t
        # unit 0 first, in 512-col pieces; then the query tail, then the rest
        for s in range(0, 2048, 512):
            for c in range(2):
                dma(dts[(0, c)][:, :, s : s + 512],
                    dq[:, 2 * c : 2 * c + 2, s : s + 512])
        dma(xt[:, 2:4, :], xq[:, 2:4, :])
        for g, (goff, gw) in enumerate(UNITS):
            if g == 0:
                continue
            for s in range(0, gw, 1024):
                w = min(1024, gw - s)
                for c in range(2):
                    dma(dts[(g, c)][:, :, s : s + w],
                        dq[:, 2 * c : 2 * c + 2, goff + s : goff + s + w])

        cvall = const.tile([128, QT * NCAND], F32, tag="cvall", name="cvall")

        sc16, sgm, sgp = [], [], []
        for i in range(NBUF):
            t = const.tile([128, NPAD], F16, tag=f"sc{i}", name=f"sc{i}")
            sc16.append(t)
            t = const.tile([128, NSEG], F16, tag=f"sgm{i}", name=f"sgm{i}")
            sgm.append(t)
            t = const.tile([128, NSEG], F32, tag=f"sgp{i}", name=f"sgp{i}")
            nc.gpsimd.iota(
                t.bitcast(U16)[:, 0 : 2 * NSEG : 2],
                pattern=[[1, NSEG]],
                base=0,
                channel_multiplier=0,
            )
            sgp.append(t)

        # PE warm-up during the DMA lead-in: ~20 junk matmuls on xt keep the
        # PE HAM busy so the real matmuls start at the 2.4 GHz clock
        for r in range(20):
            wp = ppool.tile([128, 2048], F32, tag="pp")
            nc.tensor.matmul(
                wp[:, :512], xt[:, 0:2, :128], xt[:, 0:2, :512],
                start=True, stop=True, perf_mode=DR,
            )

        for qt in range(QT):
            b = qt % NBUF
            for g, (goff, gw) in enumerate(UNITS):
                s0, s1 = goff // SEG, (goff + gw) // SEG
                pp = ppool.tile([128, 2048], F32, tag="pp")
                nsub = (gw + SUBW - 1) // SUBW
                for s in range(nsub):
                    w = min(SUBW, gw - s * SUBW)
                    out_sl = pp[:, s * SUBW : s * SUBW + w]
                    for c in range(2):
                        nc.tensor.matmul(
                            out_sl,
                            xt[:, 2 * c : 2 * c + 2, qt * 128 : (qt + 1) * 128],
                            dts[(g, c)][:, :, s * SUBW : s * SUBW + w],
                            start=(c == 0), stop=(c == 1), perf_mode=DR,
                        )
                # PSUM -> dense fp16, one read per unit
                nc.scalar.activation(
                    sc16[b][:, goff : goff + gw], pp[:, :gw], COPY
                )
                nc.vector.tensor_reduce(
                    sgm[b][:, s0:s1],
                    sc16[b][:, goff : goff + gw].rearrange(
                        "p (s e) -> p s e", e=SEG
                    ),
                    axis=AX, op=MAX,
                )
                nc.gpsimd.tensor_copy(
                    sgp[b].bitcast(F16)[:, 2 * s0 + 1 : 2 * s1 : 2],
                    sgm[b][:, s0:s1],
                )
                col = qt * NCAND + g * 8
                nc.vector.max(out=cvall[:, col : col + 8], in_=sgp[b][:, s0:s1])
            # per-qt store: only the last (tiny) slice lands in the tail
            nc.gpsimd.dma_start(
                vals_o[:, qt * NCAND : (qt + 1) * NCAND],
                cvall[:, qt * NCAND : (qt + 1) * NCAND],
            )
    if not nc.is_finalized():
        nc.finalize()
    return nc


def _prep_inputs(X: np.ndarray, data: np.ndarray) -> list[dict[str, np.ndarray]]:
    e4 = ml_dtypes.float8_e4m3fn
    Xf = X.astype(np.float64)
    # query chunks: [p, 2c+s, q]; chunk1 ksub pair carries dims 256..509 on
    # partitions 0..126 and the constant 1.0 on partition 127 (bias rows)
    xqf = np.zeros((128, 4, Q), np.float64)
    xqf[:, 0, :] = (2.0 * Xf[:, 0:128]).T
    xqf[:, 1, :] = (2.0 * Xf[:, 128:256]).T
    xqf[:127, 2, :] = (2.0 * Xf[:, 256:383]).T
    xqf[:127, 3, :] = (2.0 * Xf[:, 383:510]).T
    xqf[127, 2, :] = 1.0
    xqf[127, 3, :] = 1.0
    xq8 = xqf.astype(e4)

    in_maps = []
    for i in range(CORES):
        sh = np.asarray(data[i * NSH : (i + 1) * NSH], dtype=np.float64)
        d2 = np.einsum("nd,nd->n", sh, sh)
        bias = np.full((NPAD,), -240.0, np.float64)
        bias[:NSH] = 512.0 - d2
        b0 = bias.astype(e4)
        b1 = np.where(
            np.arange(NPAD) < NSH, bias - b0.astype(np.float64), -240.0
        ).astype(e4)
        dqf = np.zeros((128, 4, NPAD), np.float64)
        dqf[:, 0, :NSH] = sh[:, 0:128].T
        dqf[:, 1, :NSH] = sh[:, 128:256].T
        dqf[:127, 2, :NSH] = sh[:, 256:383].T
        dqf[:127, 3, :NSH] = sh[:, 383:510].T
        dq8 = dqf.astype(e4)
        dq8[127, 2, :] = b0
        dq8[127, 3, :] = b1
        in_maps.append({"xq": xq8, "dq": dq8})
    return in_maps


def _merge(results, X, data, targets) -> np.ndarray:
    def unpack(a):  # [128, QT*NCAND] -> [Q, NCAND]
        return a.reshape(128, QT, NCAND).transpose(1, 0, 2).reshape(Q, NCAND)

    packed = np.stack(
        [unpack(results[i]["vals"]).view(np.uint32) for i in range(CORES)]
    )                                                      # [CORES, Q, NCAND]
    segidx = (packed & 0xFFFF).astype(np.int64)            # segment in shard row
    segmax = (packed >> 16).astype(np.uint16).view(np.float16).astype(np.float64)
    gseg = segidx + (np.arange(CORES, dtype=np.int64) * NSEG)[:, None, None]
    allv = segmax.transpose(1, 0, 2).reshape(Q, CORES * NCAND)
    alli = gseg.transpose(1, 0, 2).reshape(Q, CORES * NCAND)

    Xd = np.asarray(X, dtype=np.float64)
    dd = np.asarray(data, dtype=np.float64)
    tgt = np.asarray(targets, dtype=np.int64)

    def seg_cols(gs):
        core, seg = divmod(int(gs), NSEG)
        base = seg * SEG
        hi = min(base + SEG, NSH)
        if base >= NSH:
            return np.empty(0, np.int64)
        return core * NSH + np.arange(base, hi, dtype=np.int64)

    P1 = 16
    order = np.argsort(-allv, axis=1)
    pred = np.empty(Q, np.float32)
    counts = np.zeros(NUM_CLASSES, np.int32)
    for q in range(Q):
        segs1 = alli[q, order[q, :P1]]
        cols = np.concatenate([seg_cols(gs) for gs in segs1])
        sq = ((dd[cols] - Xd[q]) ** 2).sum(1)
        ord1 = np.argsort(sq, kind="stable")
        t10 = sq[ord1[min(K - 1, len(sq) - 1)]]            # 10th-best dist^2
        # s_dev ~ 512 + ||x||^2 - dist^2 (+/- DELTA device error): any segment
        # whose segmax is below this cannot hold a point within t10
        x2q = (Xd[q] ** 2).sum()
        thresh = (512.0 + x2q - t10) - DELTA
        rest = order[q, P1:]
        live = rest[allv[q, rest] >= thresh]
        if len(live):
            cols2 = np.concatenate([seg_cols(gs) for gs in alli[q, live]])
            if len(cols2):
                sq2 = ((dd[cols2] - Xd[q]) ** 2).sum(1)
                cols = np.concatenate([cols, cols2])
                sq = np.concatenate([sq, sq2])
        o = np.lexsort((cols, sq))[:K]
        top10 = cols[o]
        counts[:] = 0
        np.add.at(counts, tgt[top10], 1)
        pred[q] = counts.argmax()
    return pred


def kernel(X: np.ndarray, data: np.ndarray, targets: np.ndarray) -> np.ndarray:
    X = np.asarray(X)
    data = np.asarray(data)
    targets = np.asarray(targets)
    nc = build_program()
    in_maps = _prep_inputs(X, data)
    results = run_bass_kernel_spmd(nc, in_maps, list(range(CORES))).results
    return _merge(results, X, data, targets)


if __name__ == "__main__":
    import reference

    inputs = reference.setup_inputs()
    inputs = {k: np.asarray(v) for k, v in inputs.items()}
    out = kernel(**inputs)
    print(out[:16])


# revision 22
# speedup vs baseline: 1.3188x; 1.0736x over previous
"""KNN (k=10, mode vote over 100 classes) on 8 Trainium2 cores.

Strategy: shard the reference set `data`/`targets` across 8 cores along N
(6250 rows each, padded to 6400). Each core computes, for every query q and
local point n, the score  s[q,n] = 2*X[q]@d[n] + (512 - ||d[n]||^2)  (monotone
in -dist^2 per query; +512 centers scores near 0 for fp16 fidelity).

Matmuls are fp8e4m3 DoubleRow (K=256 per instruction, streaming at the same
~217ns/512-col pace as a K=128 fp16 matmul -> 2x MAC throughput). The bias
rides inside the second contraction chunk: chunk1 = dims 0..255; chunk2 =
dims 256..509 on partitions 0..126 plus the fp8 bias and its fp8 residual on
partition 127 (query side carries 1.0 there). Dims 510/511 are dropped from
the device score (noise sigma ~2.8, audited harmless). Two matmuls per
128-query x 512-point tile.

Candidate extraction is hierarchical: ScalarE copies PSUM->SBUF as dense
fp16; VectorE tensor_reduce (2x 16-bit mode) computes the max of every
16-wide segment; GpSimd merges segment maxes into packed fp32 words
(fp16 segmax << 16 | segment index, IEEE order = lexicographic); VectorE
max8 then returns the top-8 segments of each 2048-wide unit with their
indices in one short pass. A unit's top-8 segments provably contain its
top-8 elements, and no unit holds >8 of a query's true top-10 (audited:
max 5, worst in-unit device rank 4).

Host merges 8 cores x 4 units x 8 = 256 candidate segments per query and
rescores exactly in fp64 with sound adaptive pruning: after rescoring the
top-16 segments by segmax, any unscored segment whose segmax (an upper bound
on members' device scores) is below the current 10th-best exact score minus
the device-error margin cannot hold a true top-10 point.
"""

from contextlib import ExitStack

import numpy as np
import ml_dtypes

import concourse.bacc as bacc
import concourse.bass as bass
import concourse.mybir as mybir
from concourse.bass_utils import run_bass_kernel_spmd
from concourse.tile import TileContext

F32 = mybir.dt.float32
F16 = mybir.dt.float16
FP8 = mybir.dt.float8e4
U16 = mybir.dt.uint16
COPY = mybir.ActivationFunctionType.Copy
DR = mybir.MatmulPerfMode.DoubleRow
MAX = mybir.AluOpType.max
AX = mybir.AxisListType.X

Q = 1024            # queries
D = 512             # feature dim
N = 50000           # reference points
CORES = 8
NSH = N // CORES    # 6250 per core
NPAD = 6400         # padded shard width
K = 10
NUM_CLASSES = 100
SUBW = 512          # matmul free-dim tile (one PSUM bank)
SEG = 32
NSEG = NPAD // SEG  # 200 segments per row
UNITS = [(0, 2048), (2048, 2048), (4096, 2048), (6144, 256)]
NCAND = len(UNITS) * 8   # 32 candidate segments per core per query
QT = Q // 128
NBUF = 4
DELTA = 24.0        # device-score error margin for sound host pruning


def build_program() -> bass.Bass:
    nc = bacc.Bacc()
    xq = nc.declare_dram_parameter("xq", [128, 4, Q], FP8, isOutput=False)
    dq = nc.declare_dram_parameter("dq", [128, 4, NPAD], FP8, isOutput=False)
    vals_o = nc.declare_dram_parameter("vals", [128, QT * NCAND], F32, isOutput=True)

    with TileContext(nc) as tc, ExitStack() as ctx:
        const = ctx.enter_context(tc.tile_pool(name="const", bufs=1))
        ppool = ctx.enter_context(tc.tile_pool(name="ppool", bufs=4, space="PSUM"))

        # spread input DMAs across engine DGE rings so transfers overlap;
        # order = first-use order so unit 0 computes while the rest stream in
        # input DMAs split fine-grained, issued in first-use order across the
        # three DGE rings so unit 0's data lands within ~1us of kernel start
        rings = [nc.sync, nc.scalar]
        ring_i = 0

        def dma(dst, src):
            nonlocal ring_i
            rings[ring_i % 2].dma_start(dst, src)
            ring_i += 1

        xt = const.tile([128, 4, Q], FP8, tag="xt", name="xt")
        dma(xt[:, 0:2, :], xq[:, 0:2, :])
        dts = {}
        for g, (goff, gw) in enumerate(UNITS):
            for c in range(2):
                t = const.tile([128, 2, gw], FP8, tag=f"dt{g}_{c}", name=f"dt{g}_{c}")
                dts[(g, c)] = t
        # unit 0 first, in 512-col pieces; then the query tail, then the rest
        for s in range(0, 2048, 512):
            for c in range(2):
                dma(dts[(0, c)][:, :, s : s + 512],
                    dq[:, 2 * c : 2 * c + 2, s : s + 512])
        dma(xt[:, 2:4, :], xq[:, 2:4, :])
        for g, (goff, gw) in enumerate(UNITS):
            if g == 0:
                continue
            for s in range(0, gw, 1024):
                w = min(1024, gw - s)
                for c in range(2):
                    dma(dts[(g, c)][:, :, s : s + w],
                        dq[:, 2 * c : 2 * c + 2, goff + s : goff + s + w])

        cvall = const.tile([128, QT * NCAND], F32, tag="cvall", name="cvall")

        sc16, sgp = [], []
        for i in range(NBUF):
            t = const.tile([128, NPAD], F16, tag=f"sc{i}", name=f"sc{i}")
            sc16.append(t)
            t = const.tile([128, NSEG], F32, tag=f"sgp{i}", name=f"sgp{i}")
            nc.gpsimd.iota(
                t.bitcast(U16)[:, 0 : 2 * NSEG : 2],
                pattern=[[1, NSEG]],
                base=0,
                channel_multiplier=0,
            )
            sgp.append(t)

        # PE warm-up during the DMA lead-in: ~20 junk matmuls on xt keep the
        # PE HAM busy so the real matmuls start at the 2.4 GHz clock
        for r in range(20):
            wp = ppool.tile([128, 1024], F32, tag="pp")
            nc.tensor.matmul(
                wp[:, :512], xt[:, 0:2, :128], xt[:, 0:2, :512],
                start=True, stop=True, perf_mode=DR,
            )

        for qt in range(QT):
            b = qt % NBUF
            for g, (goff, gw) in enumerate(UNITS):
                s0, s1 = goff // SEG, (goff + gw) // SEG
                # 2-bank PSUM tiles: finer WAR release to keep the PE fed
                for h in range(0, gw, 2 * SUBW):
                    hw_ = min(2 * SUBW, gw - h)
                    pp = ppool.tile([128, 1024], F32, tag="pp")
                    for s in range(0, hw_, SUBW):
                        w = min(SUBW, hw_ - s)
                        out_sl = pp[:, s : s + w]
                        for c in range(2):
                            nc.tensor.matmul(
                                out_sl,
                                xt[:, 2 * c : 2 * c + 2, qt * 128 : (qt + 1) * 128],
                                dts[(g, c)][:, :, h + s : h + s + w],
                                start=(c == 0), stop=(c == 1), perf_mode=DR,
                            )
                    nc.scalar.activation(
                        sc16[b][:, goff + h : goff + h + hw_], pp[:, :hw_], COPY
                    )
                # segment maxes written straight into the packed words' fp16
                # halves (the reduce runs at 1x regardless, strided out is free)
                nc.vector.tensor_reduce(
                    sgp[b].bitcast(F16)[:, 2 * s0 + 1 : 2 * s1 : 2],
                    sc16[b][:, goff : goff + gw].rearrange(
                        "p (s e) -> p s e", e=SEG
                    ),
                    axis=AX, op=MAX,
                )
                col = qt * NCAND + g * 8
                nc.vector.max(out=cvall[:, col : col + 8], in_=sgp[b][:, s0:s1])
            # per-qt store: only the last (tiny) slice lands in the tail
            nc.gpsimd.dma_start(
                vals_o[:, qt * NCAND : (qt + 1) * NCAND],
                cvall[:, qt * NCAND : (qt + 1) * NCAND],
            )
    if not nc.is_finalized():
        nc.finalize()
    return nc


def _prep_inputs(X: np.ndarray, data: np.ndarray) -> list[dict[str, np.ndarray]]:
    e4 = ml_dtypes.float8_e4m3fn
    Xf = X.astype(np.float64)
    # query chunks: [p, 2c+s, q]; chunk1 ksub pair carries dims 256..509 on
    # partitions 0..126 and the constant 1.0 on partition 127 (bias rows)
    xqf = np.zeros((128, 4, Q), np.float64)
    xqf[:, 0, :] = (2.0 * Xf[:, 0:128]).T
    xqf[:, 1, :] = (2.0 * Xf[:, 128:256]).T
    xqf[:127, 2, :] = (2.0 * Xf[:, 256:383]).T
    xqf[:127, 3, :] = (2.0 * Xf[:, 383:510]).T
    xqf[127, 2, :] = 1.0
    xqf[127, 3, :] = 1.0
    xq8 = xqf.astype(e4)

    in_maps = []
    for i in range(CORES):
        sh = np.asarray(data[i * NSH : (i + 1) * NSH], dtype=np.float64)
        d2 = np.einsum("nd,nd->n", sh, sh)
        bias = np.full((NPAD,), -240.0, np.float64)
        bias[:NSH] = 512.0 - d2
        b0 = bias.astype(e4)
        b1 = np.where(
            np.arange(NPAD) < NSH, bias - b0.astype(np.float64), -240.0
        ).astype(e4)
        dqf = np.zeros((128, 4, NPAD), np.float64)
        dqf[:, 0, :NSH] = sh[:, 0:128].T
        dqf[:, 1, :NSH] = sh[:, 128:256].T
        dqf[:127, 2, :NSH] = sh[:, 256:383].T
        dqf[:127, 3, :NSH] = sh[:, 383:510].T
        dq8 = dqf.astype(e4)
        dq8[127, 2, :] = b0
        dq8[127, 3, :] = b1
        in_maps.append({"xq": xq8, "dq": dq8})
    return in_maps


def _merge(results, X, data, targets) -> np.ndarray:
    def unpack(a):  # [128, QT*NCAND] -> [Q, NCAND]
        return a.reshape(128, QT, NCAND).transpose(1, 0, 2).reshape(Q, NCAND)

    packed = np.stack(
        [unpack(results[i]["vals"]).view(np.uint32) for i in range(CORES)]
    )                                                      # [CORES, Q, NCAND]
    segidx = (packed & 0xFFFF).astype(np.int64)            # segment in shard row
    segmax = (packed >> 16).astype(np.uint16).view(np.float16).astype(np.float64)
    gseg = segidx + (np.arange(CORES, dtype=np.int64) * NSEG)[:, None, None]
    allv = segmax.transpose(1, 0, 2).reshape(Q, CORES * NCAND)
    alli = gseg.transpose(1, 0, 2).reshape(Q, CORES * NCAND)

    Xd = np.asarray(X, dtype=np.float64)
    dd = np.asarray(data, dtype=np.float64)
    tgt = np.asarray(targets, dtype=np.int64)

    def seg_cols(gs):
        core, seg = divmod(int(gs), NSEG)
        base = seg * SEG
        hi = min(base + SEG, NSH)
        if base >= NSH:
            return np.empty(0, np.int64)
        return core * NSH + np.arange(base, hi, dtype=np.int64)

    P1 = 16
    order = np.argsort(-allv, axis=1)
    pred = np.empty(Q, np.float32)
    counts = np.zeros(NUM_CLASSES, np.int32)
    for q in range(Q):
        segs1 = alli[q, order[q, :P1]]
        cols = np.concatenate([seg_cols(gs) for gs in segs1])
        sq = ((dd[cols] - Xd[q]) ** 2).sum(1)
        ord1 = np.argsort(sq, kind="stable")
        t10 = sq[ord1[min(K - 1, len(sq) - 1)]]            # 10th-best dist^2
        # s_dev ~ 512 + ||x||^2 - dist^2 (+/- DELTA device error): any segment
        # whose segmax is below this cannot hold a point within t10
        x2q = (Xd[q] ** 2).sum()
        thresh = (512.0 + x2q - t10) - DELTA
        rest = order[q, P1:]
        live = rest[allv[q, rest] >= thresh]
        if len(live):
            cols2 = np.concatenate([seg_cols(gs) for gs in alli[q, live]])
            if len(cols2):
                sq2 = ((dd[cols2] - Xd[q]) ** 2).sum(1)
                cols = np.concatenate([cols, cols2])
                sq = np.concatenate([sq, sq2])
        o = np.lexsort((cols, sq))[:K]
        top10 = cols[o]
        counts[:] = 0
        np.add.at(counts, tgt[top10], 1)
        pred[q] = counts.argmax()
    return pred


def kernel(X: np.ndarray, data: np.ndarray, targets: np.ndarray) -> np.ndarray:
    X = np.asarray(X)
    data = np.asarray(data)
    targets = np.asarray(targets)
    nc = build_program()
    in_maps = _prep_inputs(X, data)
    results = run_bass_kernel_spmd(nc, in_maps, list(range(CORES))).results
    return _merge(results, X, data, targets)


if __name__ == "__main__":
    import reference

    inputs = reference.setup_inputs()
    inputs = {k: np.asarray(v) for k, v in inputs.items()}
    out = kernel(**inputs)
    print(out[:16])


# revision 24
# speedup vs baseline: 1.3375x; 1.0142x over previous
"""KNN (k=10, mode vote over 100 classes) on 8 Trainium2 cores.

Strategy: shard the reference set `data`/`targets` across 8 cores along N
(6250 rows each, padded to 6400). Each core computes, for every query q and
local point n, the score  s[q,n] = 2*X[q]@d[n] + (512 - ||d[n]||^2)  (monotone
in -dist^2 per query; +512 centers scores near 0 for fp16 fidelity).

Matmuls are fp8e4m3 DoubleRow (K=256 per instruction, streaming at the same
~217ns/512-col pace as a K=128 fp16 matmul -> 2x MAC throughput). The bias
rides inside the second contraction chunk: chunk1 = dims 0..255; chunk2 =
dims 256..509 on partitions 0..126 plus the fp8 bias and its fp8 residual on
partition 127 (query side carries 1.0 there). Dims 510/511 are dropped from
the device score (noise sigma ~2.8, audited harmless). Two matmuls per
128-query x 512-point tile.

Candidate extraction is hierarchical: ScalarE copies PSUM->SBUF as dense
fp16 (2 banks per instruction, finest PSUM WAR release); VectorE
tensor_reduce computes the max of every 32-wide segment, writing the fp16
result straight into the odd uint16 halves of fp32 "packed words" whose even
halves hold a one-time GpSimd iota of segment indices (IEEE fp32 order =
(segmax, segidx) lexicographic order); VectorE max8 then returns the top-8
segments of each 2048-wide unit with their indices in one short pass. A
unit's top-8 segments provably contain its top-8 elements, and no unit
holds >8 of a query's true top-10 (audited: max 5, worst in-unit device
rank 4).

Host merges 8 cores x 4 units x 8 = 256 candidate segments per query and
rescores exactly in fp64 with sound adaptive pruning: after rescoring the
top-16 segments by segmax, any unscored segment whose segmax (an upper bound
on members' device scores) is below the current 10th-best exact score minus
the device-error margin cannot hold a true top-10 point.
"""

from contextlib import ExitStack

import numpy as np
import ml_dtypes

import concourse.bacc as bacc
import concourse.bass as bass
import concourse.mybir as mybir
from concourse.bass_utils import run_bass_kernel_spmd
from concourse.tile import TileContext

F32 = mybir.dt.float32
F16 = mybir.dt.float16
FP8 = mybir.dt.float8e4
U16 = mybir.dt.uint16
COPY = mybir.ActivationFunctionType.Copy
DR = mybir.MatmulPerfMode.DoubleRow
MAX = mybir.AluOpType.max
AX = mybir.AxisListType.X

Q = 1024            # queries
D = 512             # feature dim
N = 50000           # reference points
CORES = 8
NSH = N // CORES    # 6250 per core
NPAD = 6400         # padded shard width
K = 10
NUM_CLASSES = 100
SUBW = 512          # matmul free-dim tile (one PSUM bank)
SEG = 32
NSEG = NPAD // SEG  # 200 segments per row
UNITS = [(0, 2048), (2048, 2048), (4096, 2048), (6144, 256)]
NCAND = len(UNITS) * 8   # 32 candidate segments per core per query
QT = Q // 128
NBUF = 4
DELTA = 24.0        # device-score error margin for sound host pruning


def build_program() -> bass.Bass:
    nc = bacc.Bacc()
    xq = nc.declare_dram_parameter("xq", [128, 4, Q], FP8, isOutput=False)
    dq = nc.declare_dram_parameter("dq", [128, 4, NPAD], FP8, isOutput=False)
    vals_o = nc.declare_dram_parameter("vals", [128, QT * NCAND], F32, isOutput=True)

    with TileContext(nc) as tc, ExitStack() as ctx:
        const = ctx.enter_context(tc.tile_pool(name="const", bufs=1))
        ppool = ctx.enter_context(tc.tile_pool(name="ppool", bufs=4, space="PSUM"))

        # input DMAs split fine-grained on the two HWDGE rings, issued in
        # first-use order so unit 0 computes while the rest stream in
        rings = [nc.sync, nc.scalar]
        ring_i = 0

        def dma(dst, src):
            nonlocal ring_i
            rings[ring_i % 2].dma_start(dst, src)
            ring_i += 1

        xt = const.tile([128, 4, Q], FP8, tag="xt", name="xt")
        dma(xt[:, 0:2, :], xq[:, 0:2, :])
        dts = {}
        for g, (goff, gw) in enumerate(UNITS):
            for c in range(2):
                t = const.tile([128, 2, gw], FP8, tag=f"dt{g}_{c}", name=f"dt{g}_{c}")
                dts[(g, c)] = t
        # unit 0 first, in 512-col pieces; then the query tail, then the rest
        for s in range(0, 2048, 512):
            for c in range(2):
                dma(dts[(0, c)][:, :, s : s + 512],
                    dq[:, 2 * c : 2 * c + 2, s : s + 512])
        dma(xt[:, 2:4, :], xq[:, 2:4, :])
        for g, (goff, gw) in enumerate(UNITS):
            if g == 0:
                continue
            for s in range(0, gw, 1024):
                w = min(1024, gw - s)
                for c in range(2):
                    dma(dts[(g, c)][:, :, s : s + w],
                        dq[:, 2 * c : 2 * c + 2, goff + s : goff + s + w])

        cvall = const.tile([128, QT * NCAND], F32, tag="cvall", name="cvall")

        sc16, sgp = [], []
        for i in range(NBUF):
            t = const.tile([128, NPAD], F16, tag=f"sc{i}", name=f"sc{i}")
            sc16.append(t)
            t = const.tile([128, NSEG], F32, tag=f"sgp{i}", name=f"sgp{i}")
            nc.gpsimd.iota(
                t.bitcast(U16)[:, 0 : 2 * NSEG : 2],
                pattern=[[1, NSEG]],
                base=0,
                channel_multiplier=0,
            )
            sgp.append(t)

        # PE warm-up during the DMA lead-in: ~20 junk matmuls on xt keep the
        # PE HAM busy so the real matmuls start at the 2.4 GHz clock
        for r in range(20):
            wp = ppool.tile([128, 1024], F32, tag="pp")
            nc.tensor.matmul(
                wp[:, :512], xt[:, 0:2, :128], xt[:, 0:2, :512],
                start=True, stop=True, perf_mode=DR,
            )

        for qt in range(QT):
            b = qt % NBUF
            for g, (goff, gw) in enumerate(UNITS):
                s0, s1 = goff // SEG, (goff + gw) // SEG
                # 2-bank PSUM tiles: finer WAR release to keep the PE fed
                for h in range(0, gw, 2 * SUBW):
                    hw_ = min(2 * SUBW, gw - h)
                    pp = ppool.tile([128, 1024], F32, tag="pp")
                    for s in range(0, hw_, SUBW):
                        w = min(SUBW, hw_ - s)
                        out_sl = pp[:, s : s + w]
                        for c in range(2):
                            nc.tensor.matmul(
                                out_sl,
                                xt[:, 2 * c : 2 * c + 2, qt * 128 : (qt + 1) * 128],
                                dts[(g, c)][:, :, h + s : h + s + w],
                                start=(c == 0), stop=(c == 1), perf_mode=DR,
                            )
                    nc.scalar.activation(
                        sc16[b][:, goff + h : goff + h + hw_], pp[:, :hw_], COPY
                    )
                # segment maxes written straight into the packed words' fp16
                # halves (the reduce runs at 1x regardless, strided out is free)
                nc.vector.tensor_reduce(
                    sgp[b].bitcast(F16)[:, 2 * s0 + 1 : 2 * s1 : 2],
                    sc16[b][:, goff : goff + gw].rearrange(
                        "p (s e) -> p s e", e=SEG
                    ),
                    axis=AX, op=MAX,
                )
                col = qt * NCAND + g * 8
                nc.vector.max(out=cvall[:, col : col + 8], in_=sgp[b][:, s0:s1])
            # per-qt store: only the last (tiny) slice lands in the tail
            nc.gpsimd.dma_start(
                vals_o[:, qt * NCAND : (qt + 1) * NCAND],
                cvall[:, qt * NCAND : (qt + 1) * NCAND],
            )
    if not nc.is_finalized():
        nc.finalize()
    return nc


def _prep_inputs(X: np.ndarray, data: np.ndarray) -> list[dict[str, np.ndarray]]:
    e4 = ml_dtypes.float8_e4m3fn
    Xf = X.astype(np.float64)
    # query chunks: [p, 2c+s, q]; chunk1 ksub pair carries dims 256..509 on
    # partitions 0..126 and the constant 1.0 on partition 127 (bias rows)
    xqf = np.zeros((128, 4, Q), np.float64)
    xqf[:, 0, :] = (2.0 * Xf[:, 0:128]).T
    xqf[:, 1, :] = (2.0 * Xf[:, 128:256]).T
    xqf[:127, 2, :] = (2.0 * Xf[:, 256:383]).T
    xqf[:127, 3, :] = (2.0 * Xf[:, 383:510]).T
    xqf[127, 2, :] = 1.0
    xqf[127, 3, :] = 1.0
    xq8 = xqf.astype(e4)

    in_maps = []
    for i in range(CORES):
        sh = np.asarray(data[i * NSH : (i + 1) * NSH], dtype=np.float64)
        d2 = np.einsum("nd,nd->n", sh, sh)
        bias = np.full((NPAD,), -240.0, np.float64)
        bias[:NSH] = 512.0 - d2
        b0 = bias.astype(e4)
        b1 = np.where(
            np.arange(NPAD) < NSH, bias - b0.astype(np.float64), -240.0
        ).astype(e4)
        dqf = np.zeros((128, 4, NPAD), np.float64)
        dqf[:, 0, :NSH] = sh[:, 0:128].T
        dqf[:, 1, :NSH] = sh[:, 128:256].T
        dqf[:127, 2, :NSH] = sh[:, 256:383].T
        dqf[:127, 3, :NSH] = sh[:, 383:510].T
        dq8 = dqf.astype(e4)
        dq8[127, 2, :] = b0
        dq8[127, 3, :] = b1
        in_maps.append({"xq": xq8, "dq": dq8})
    return in_maps


def _merge(results, X, data, targets) -> np.ndarray:
    def unpack(a):  # [128, QT*NCAND] -> [Q, NCAND]
        return a.reshape(128, QT, NCAND).transpose(1, 0, 2).reshape(Q, NCAND)

    packed = np.stack(
        [unpack(results[i]["vals"]).view(np.uint32) for i in range(CORES)]
    )                                                      # [CORES, Q, NCAND]
    segidx = (packed & 0xFFFF).astype(np.int64)            # segment in shard row
    segmax = (packed >> 16).astype(np.uint16).view(np.float16).astype(np.float64)
    gseg = segidx + (np.arange(CORES, dtype=np.int64) * NSEG)[:, None, None]
    allv = segmax.transpose(1, 0, 2).reshape(Q, CORES * NCAND)
    alli = gseg.transpose(1, 0, 2).reshape(Q, CORES * NCAND)

    Xd = np.asarray(X, dtype=np.float64)
    dd = np.asarray(data, dtype=np.float64)
    tgt = np.asarray(targets, dtype=np.int64)

    def seg_cols(gs):
        core, seg = divmod(int(gs), NSEG)
        base = seg * SEG
        hi = min(base + SEG, NSH)
        if base >= NSH:
            return np.empty(0, np.int64)
        return core * NSH + np.arange(base, hi, dtype=np.int64)

    P1 = 16
    order = np.argsort(-allv, axis=1)
    pred = np.empty(Q, np.float32)
    counts = np.zeros(NUM_CLASSES, np.int32)
    for q in range(Q):
        segs1 = alli[q, order[q, :P1]]
        cols = np.concatenate([seg_cols(gs) for gs in segs1])
        sq = ((dd[cols] - Xd[q]) ** 2).sum(1)
        ord1 = np.argsort(sq, kind="stable")
        t10 = sq[ord1[min(K - 1, len(sq) - 1)]]            # 10th-best dist^2
        # s_dev ~ 512 + ||x||^2 - dist^2 (+/- DELTA device error): any segment
        # whose segmax is below this cannot hold a point within t10
        x2q = (Xd[q] ** 2).sum()
        thresh = (512.0 + x2q - t10) - DELTA
        rest = order[q, P1:]
        live = rest[allv[q, rest] >= thresh]
        if len(live):
            cols2 = np.concatenate([seg_cols(gs) for gs in alli[q, live]])
            if len(cols2):
                sq2 = ((dd[cols2] - Xd[q]) ** 2).sum(1)
                cols = np.concatenate([cols, cols2])
                sq = np.concatenate([sq, sq2])
        o = np.lexsort((cols, sq))[:K]
        top10 = cols[o]
        counts[:] = 0
        np.add.at(counts, tgt[top10], 1)
        pred[q] = counts.argmax()
    return pred


def kernel(X: np.ndarray, data: np.ndarray, targets: np.ndarray) -> np.ndarray:
    X = np.asarray(X)
    data = np.asarray(data)
    targets = np.asarray(targets)
    nc = build_program()
    in_maps = _prep_inputs(X, data)
    results = run_bass_kernel_spmd(nc, in_maps, list(range(CORES))).results
    return _merge(results, X, data, targets)


if __name__ == "__main__":
    import reference

    inputs = reference.setup_inputs()
    inputs = {k: np.asarray(v) for k, v in inputs.items()}
    out = kernel(**inputs)
    print(out[:16])
